# revision 19
# baseline (speedup 1.0000x reference)
"""Trainium2 Bass kernel for nn_MultiHeadAttention_53815940219243.

Reference computation (single-head attention with full 1024-dim contraction):
    q = x @ Wq + bq; k = x @ Wk + bk; v = x @ Wv + bv        # [4096, 1024]
    scores = softmax(q @ k.T, axis=-1) / sqrt(64)            # [4096, 4096]
    z = scores @ v                                           # [4096, 1024]
    out = z @ Wo + bo                                        # [4096, 64]

Algebraic restructure (all weight-only products precomputed on host):
  * softmax is shift-invariant per row, so bk and the bq.(x Wq)-row term drop:
        S_eff[i,j] = x_i (Wq Wk^T) x_j^T + c_j,   c = x @ (Wk @ bq)
    With A = Wq Wk^T:  B = x A  (the only "Q/K projection" left), and the
    "K" operand of the score matmul is x itself -> no K projection and NO
    collectives at all (x is replicated; each core computes its 512 score
    columns).
  * exp(S + c) = exp(S) * exp(c): the per-key factor exp(c_j) is folded into
    the value rows, so the score phase is a pure matmul + Exp activation.
  * v@Wo is folded on host: vw = x @ (Wv Wo) + bv Wo  [4096, 64], and the
    softmax denominator comes free as a ones-column appended to vw:
        num[:, 0:64] = E^T @ (vw/8 * expc),  num[:, 64] = E^T @ expc = den/8?
    (the 1/8 = 1/sqrt(64) is folded into the vw columns only, so
     out = num[:,0:64] / num[:,64] + bo.)
  * No rank-1 bias matmuls anywhere (LDWEIGHTS-expensive): biases are added
    by DVE/ACT per-partition ops instead.

Dataflow per core (shard = 512 query rows; transposed score space, fp16
matmul operands, f32 PSUM accumulation; E stored bf16 since exp() can reach
e^60):
    B-proj: bt[dout, qi]   = sum_c a16[:,c,dout-tile]^T @ xq[:,c,:]    (fp16)
    S:      st[kj,  qi]    = sum_c xt[:,c,kj-tile]^T @ bt[:,c,:]       (fp16)
            et = Exp(st)   (one Exp instruction per PAIR of tiles: ACT
            costs (N+352)/1.2ns per instruction)                       (bf16)
    V:      vt[h', kj]     = sum_c wvo_ext[:,c,:]^T @ xt-chunk  (66 rows:
            64 value cols + denominator-ones row + c row; N=512 matmuls
            with wvo stationary -- the kj-on-partitions orientation would
            need 256 small matmuls, each paying the full ~107ns LDWEIGHTS)
            PE-transpose 128-col tiles -> [kj, 66]; expc = Exp(col 65);
            vw = (vt + bvo) * expc via DVE per-partition scale        (bf16)
    O:      o[h', qi]      = sum_jt vw[:,jt,:]^T @ et[:,jt,:]   (h'=65 rows)
            transpose 128-col tiles -> [qi, 65]; out = o[:,0:64]/o[:,64] + bo

Measurement notes (shared brokered TRN2; per-call dispatch floor ~11ms with
heavy drift): every matmul pays a ~107ns LDWEIGHTS and the port serializes
them, so many-small-matmul phases run at ~107ns/matmul regardless of N.
"""

import numpy as np

N = 4096
D = 1024
H = 64
HP = H + 1      # 65: value cols + denominator ones-column
HE = H + 2      # 66: + c column (vw projection only)
NCORES = 8
NSH = N // NCORES   # 512 query rows per core
P = 128
DT = D // P         # 8 contraction chunks over the 1024 feature dim
JT = N // P         # 32 key tiles
IT = NSH // P       # 4 output row tiles per core

_CACHE = {}

# kept for test.py compatibility (all matmuls are fp16-operand regardless)
S_FP16 = True
PROJ_FP16 = True

# interleave the V-projection matmuls into the S loop so they share the
# already-loaded stationary operand (xt tile) with the S matmuls
SV_FUSED = True


def _build(upto="O", rep_a=1, rep_ag=1, rep_s=1, rep_u=1, rep_o=1,
           s_fp16=True, proj_fp16=True, sv_fused=None):
    if sv_fused is None:
        sv_fused = SV_FUSED
    import concourse.mybir as mybir
    import concourse.tile as tile
    from concourse import bacc
    from contextlib import ExitStack

    f32 = mybir.dt.float32
    bf16 = mybir.dt.bfloat16
    fp16 = mybir.dt.float16

    nc = bacc.Bacc("TRN2", target_bir_lowering=False, num_devices=NCORES)

    # ---- kernel I/O (per core; everything but xq is replicated) ----
    xt = nc.dram_tensor("xt", [P, DT, N], fp16, kind="ExternalInput")
    xq = nc.dram_tensor("xq", [P, DT, NSH], fp16, kind="ExternalInput")
    a16 = nc.dram_tensor("a16", [P, DT, D], fp16, kind="ExternalInput")
    wvo_e = nc.dram_tensor("wvo_e", [P, DT, HE], fp16, kind="ExternalInput")
    bvo_c = nc.dram_tensor("bvo_c", [HE, 1], f32, kind="ExternalInput")
    bo_bc = nc.dram_tensor("bo_bc", [P, H], f32, kind="ExternalInput")
    ident = nc.dram_tensor("ident66", [HE, HE], f32, kind="ExternalInput")

    out = nc.dram_tensor("out", [NSH, H], f32, kind="ExternalOutput")

    with tile.TileContext(nc) as tc, ExitStack() as ctx:
        persist = ctx.enter_context(tc.tile_pool(name="persist", bufs=1))
        pp_big = ctx.enter_context(tc.tile_pool(name="pp_big", bufs=3, space="PSUM"))
        pp_sml = ctx.enter_context(tc.tile_pool(name="pp_sml", bufs=2, space="PSUM"))
        scratch = ctx.enter_context(tc.tile_pool(name="scratch", bufs=3))

        xt_sb = persist.tile([P, DT, N], fp16, tag="xt")
        xq_sb = persist.tile([P, DT, NSH], fp16, tag="xq")
        a_sb = persist.tile([P, DT, D], fp16, tag="a16")
        bt_sb = persist.tile([P, DT, NSH], fp16, tag="bt")
        et_sb = persist.tile([P, JT, NSH], bf16, tag="et")      # 32KB/part
        vw_sb = persist.tile([P, JT, HP], bf16, tag="vw")
        expc_sb = persist.tile([P, JT], f32, tag="expc")
        wvo_sb = persist.tile([P, DT, HE], fp16, tag="wvo")
        bvo_sb = persist.tile([HE, 1], f32, tag="bvo")
        bo_sb = persist.tile([P, H], f32, tag="bo")
        id_sb = persist.tile([HE, HE], f32, tag="ident")

        # ---- input DMAs (issued up front; Tile tracks readiness) ----
        nc.sync.dma_start(out=xq_sb[:], in_=xq[:, :, :])
        for m in range(DT):
            nc.sync.dma_start(out=a_sb[:, :, m * P:(m + 1) * P],
                              in_=a16[:, :, m * P:(m + 1) * P])
        nc.sync.dma_start(out=wvo_sb[:], in_=wvo_e[:, :, :])
        nc.sync.dma_start(out=bvo_sb[:], in_=bvo_c[:, :])
        nc.sync.dma_start(out=bo_sb[:], in_=bo_bc[:, :])
        nc.sync.dma_start(out=id_sb[:], in_=ident[:, :])
        for k in range(DT):
            nc.sync.dma_start(out=xt_sb[:, :, k * NSH:(k + 1) * NSH],
                              in_=xt[:, :, k * NSH:(k + 1) * NSH])

        # ---------------- phase B: bt = A^T-contraction with x shard ------
        for _r in range(rep_a):
            for m in range(DT):
                ps = pp_big.tile([P, NSH], f32, tag="ps")
                for c in range(DT):
                    nc.tensor.matmul(ps[:], a_sb[:, c, m * P:(m + 1) * P],
                                     xq_sb[:, c, :],
                                     start=(c == 0), stop=(c == DT - 1))
                nc.vector.tensor_copy(out=bt_sb[:, m, :], in_=ps[:])

        # ---------------- phase S: et = exp(x^T-tiles . bt) ---------------
        # every matmul pays a ~107ns LDWEIGHTS; N=512 streams ~107ns, so the
        # phase runs at ~107ns/matmul either way. Small-N matmuls here would
        # still cost ~107ns each (LDW-port bound) -- keep this loop pure.
        pair_exp = upto != "SNOPAIR"
        for _r in range(rep_s):
            for jp in range(JT // 2 if pair_exp else JT):
                if pair_exp:
                    # two score tiles share one psum allocation and ONE Exp
                    # instruction: ACT costs (N+352)/1.2ns per instruction,
                    # so halving the instruction count keeps ACT off the
                    # critical path
                    ps = pp_big.tile([P, 2, NSH], f32, tag="ps")
                    for h in range(2):
                        jt = jp * 2 + h
                        for c in range(DT):
                            nc.tensor.matmul(
                                ps[:, h, :], xt_sb[:, c, jt * P:(jt + 1) * P],
                                bt_sb[:, c, :],
                                start=(c == 0), stop=(c == DT - 1))
                    if upto == "SPROBE":
                        nc.vector.tensor_copy(out=et_sb[:, jp * 2:jp * 2 + 2, :],
                                              in_=ps[:, :, :])
                    else:
                        nc.scalar.activation(
                            out=et_sb[:, jp * 2:jp * 2 + 2, :], in_=ps[:, :, :],
                            func=mybir.ActivationFunctionType.Exp)
                else:
                    jt = jp
                    ps = pp_big.tile([P, NSH], f32, tag="psl")
                    for c in range(DT):
                        nc.tensor.matmul(ps[:], xt_sb[:, c, jt * P:(jt + 1) * P],
                                         bt_sb[:, c, :],
                                         start=(c == 0), stop=(c == DT - 1))
                    nc.scalar.activation(out=et_sb[:, jt, :], in_=ps[:],
                                         func=mybir.ActivationFunctionType.Exp)

        # ------- phase V: vw^T chunks (wvo stationary), PE-transpose ------
        # vt[h',kj] = sum_c wvo[:,c,:]^T @ xt-chunk: 64 N=512 matmuls instead
        # of 256 N=66 ones (which would each pay the full LDW anyway), then
        # transpose each 128-col tile back to kj-on-partitions and apply
        # bias + exp(c) scaling.
        for _r in range(rep_ag):
            for k in range(DT):
                psvt = pp_big.tile([HE, NSH], f32, tag="ps")
                for c in range(DT):
                    nc.tensor.matmul(psvt[:], wvo_sb[:, c, :],
                                     xt_sb[:, c, k * NSH:(k + 1) * NSH],
                                     start=(c == 0), stop=(c == DT - 1))
                vt_sb = scratch.tile([HE, NSH], f32, tag="vt")
                nc.vector.tensor_scalar_add(out=vt_sb[:], in0=psvt[:],
                                            scalar1=bvo_sb[:, 0:1])
                for i in range(NSH // P):
                    jt = k * (NSH // P) + i
                    pst2 = pp_sml.tile([P, HE], f32, tag="psv")
                    nc.tensor.transpose(pst2[:], vt_sb[:, i * P:(i + 1) * P],
                                        id_sb[:])
                    nc.scalar.activation(out=expc_sb[:, jt:jt + 1],
                                         in_=pst2[:, HP:HE],
                                         func=mybir.ActivationFunctionType.Exp)
                    # per-partition scale on DVE (ACT costs (N+352)/1.2ns per
                    # instruction and would become the phase bottleneck)
                    nc.vector.tensor_scalar_mul(out=vw_sb[:, jt, :],
                                                in0=pst2[:, 0:HP],
                                                scalar1=expc_sb[:, jt:jt + 1])

        # -------- phase O: o = vw''^T @ E^T; transpose; divide; + bo ------
        for _r in range(rep_o):
            pso = pp_big.tile([HP, NSH], f32, tag="ps")
            for jt in range(JT):
                nc.tensor.matmul(pso[:], vw_sb[:, jt, :], et_sb[:, jt, :],
                                 start=(jt == 0), stop=(jt == JT - 1))
            osb = scratch.tile([HP, NSH], f32, tag="osb")
            nc.vector.tensor_copy(out=osb[:], in_=pso[:])
            for qt in range(IT):
                pst = pp_sml.tile([P, HP], f32, tag="psv")
                nc.tensor.transpose(pst[:], osb[:, qt * P:(qt + 1) * P],
                                    id_sb[0:HP, 0:HP])
                r = scratch.tile([P, 1], f32, tag="rcp")
                nc.vector.reciprocal(out=r[:], in_=pst[:, H:HP])
                o_f = scratch.tile([P, H], f32, tag="ofin")
                nc.vector.scalar_tensor_tensor(
                    out=o_f[:], in0=pst[:, 0:H], scalar=r[:, 0:1], in1=bo_sb[:],
                    op0=mybir.AluOpType.mult, op1=mybir.AluOpType.add)
                nc.sync.dma_start(out=out[qt * P:(qt + 1) * P, :], in_=o_f[:])

    nc.finalize()
    return nc


def _prep_in_maps(x, Wq, bq, Wk, bk, Wv, bv, Wo, bo, proj_fp16=True):
    f32, f64 = np.float32, np.float64
    x = np.ascontiguousarray(x, dtype=f32)

    A = (np.asarray(Wq, f64) @ np.asarray(Wk, f64).T).astype(f32)
    wkbq = (np.asarray(Wk, f64) @ np.asarray(bq, f64)).astype(f32)
    wvo8 = ((np.asarray(Wv, f64) @ np.asarray(Wo, f64)) / 8.0).astype(f32)
    bvo8 = ((np.asarray(bv, f64) @ np.asarray(Wo, f64)) / 8.0).astype(f32)

    def dmaj(a):  # [1024(=c*128+p), F] -> [p, c, F] contiguous
        F = a.shape[1]
        return np.ascontiguousarray(
            a.reshape(DT, P, F).transpose(1, 0, 2)).astype(np.float16)

    xt16 = dmaj(x.T)                       # [128, 8, 4096]
    a16 = dmaj(A)                          # [128, 8, 1024]
    w66 = np.concatenate(
        [wvo8, np.zeros((D, 1), f32), wkbq[:, None]], axis=1)
    wvo_e = dmaj(w66)                      # [128, 8, 66]

    bvo_c = np.concatenate(
        [bvo8, np.array([1.0], f32), np.array([0.0], f32)]).reshape(HE, 1)
    bo_bc = np.ascontiguousarray(
        np.broadcast_to(np.asarray(bo, f32), (P, H))).astype(f32)
    ident = np.eye(HE, dtype=f32)

    shared = {
        "xt": xt16, "a16": a16, "wvo_e": wvo_e,
        "bvo_c": bvo_c, "bo_bc": bo_bc, "ident66": ident,
    }
    in_maps = []
    for c in range(NCORES):
        m = dict(shared)
        m["xq"] = np.ascontiguousarray(xt16[:, :, c * NSH:(c + 1) * NSH])
        in_maps.append(m)
    return in_maps


def kernel(x, Wq, bq, Wk, bk, Wv, bv, Wo, bo):
    import hashlib
    import os
    from concourse.bass_utils import run_bass_kernel_spmd

    key = ("nc", S_FP16, PROJ_FP16)
    if key not in _CACHE:
        _CACHE[key] = _build(s_fp16=S_FP16, proj_fp16=PROJ_FP16)
    nc = _CACHE[key]

    # the neuron disk cache keys NEFFs on an HLO fingerprint that ignores
    # the embedded Bass program; namespace the cache by the BIR hash so a
    # stale NEFF from a different kernel build can never be picked up
    digest = hashlib.sha256(nc.to_json_bytes()).hexdigest()[:16]
    os.environ["NEURON_COMPILE_CACHE_URL"] = f"/var/tmp/bass-neff-{digest}"

    in_maps = _prep_in_maps(x, Wq, bq, Wk, bk, Wv, bv, Wo, bo,
                            proj_fp16=PROJ_FP16)
    res = run_bass_kernel_spmd(nc, in_maps, core_ids=list(range(NCORES)))
    _CACHE["last_result"] = res
    return np.concatenate([res.results[c]["out"] for c in range(NCORES)], axis=0)
